# revision 25
# baseline (speedup 1.0000x reference)
"""Trainium2 Bass kernel for nn_CPRLinearFused (quantized linear).

Computes out = x @ dequant(weight_int8, scales) + bias where weights are
int8 with per-group (group=128 along K) per-output-channel scales.

Strategy:
  - Host: dequantize W to fp16 (int8 values * fp32 scales, rounded to
    fp16), transpose x to xT [K, M] fp16.
  - Device (8 NeuronCores, column-parallel over N): each core runs an
    fp16 GEMM  out_slice[M, N/8] = xT.T @ W_slice  accumulated in fp32
    PSUM, via the tuned matmul_tile_kernel (PE runs fp16 at the same
    78.6 TF/s rate as bf16, with 3 more mantissa bits).
  - Host: gather column slices, add bias in fp32.
"""

from contextlib import ExitStack

import numpy as np

import concourse.bass as bass
import concourse.mybir as mybir
import concourse.tile as tile
from concourse.bass import ts
from concourse.bass_utils import BassKernelResults, run_bass_kernel_spmd
from concourse.kernels.tile_matmul import (
    ShapeInfo,
    composable_matmul_tile_kernel,
    k_pool_min_bufs,
    matmul_tile_kernel,
)

B, S, K, N = 8, 64, 8192, 16384
M = B * S  # 512
GROUP = 128
G = K // GROUP  # 64
NCORES = 8
NSH = N // NCORES  # 2048 output columns per core

_NC = None
LAST_RESULTS = None  # BassKernelResults of the most recent run (for profiling)
LAST_IN_MAPS = None  # per-core input maps of the most recent run (for benching)


_MAX_SYNC_WAITS = 4  # this walrus build rejects >4 sync waits per instruction
_MAX_SYNC_WAITS_DMA = 1  # and >1 on DMA pseudo-instructions


def _split_sync_waits(nc):
    """Split instructions carrying more than max_waits sem waits.

    The neuronxcc walrus in this container errors with "Too many sync wait
    commands" when one instruction waits on >4 semaphores (Tile's terminal
    drain waits on ~11).  Waiting is sequential per engine sequencer, so
    hoisting the excess waits onto no-ops directly before the instruction is
    semantically identical.
    """
    counter = [0]
    for b in nc.m.functions[0].blocks:
        new_insts = []
        for inst in b.instructions:
            max_waits = _MAX_SYNC_WAITS_DMA  # 1 everywhere: engine limits vary
            si = inst.sync_info
            if si is not None and si.on_wait and len(si.on_wait) > max_waits:
                waits = list(si.on_wait)
                chunks = [
                    waits[i : i + max_waits] for i in range(0, len(waits), max_waits)
                ]
                for chunk in chunks[:-1]:
                    counter[0] += 1
                    nop = mybir.InstNoOp(
                        name=f"split_wait_nop_{counter[0]}",
                        engine=inst.engine,
                        sync_info=mybir.SyncInfo(on_wait=chunk, on_update=[]),
                    )
                    new_insts.append(nop)
                si.on_wait = chunks[-1]
            new_insts.append(inst)
        b.instructions[:] = new_insts


def _gemm_body(nc, tc, xT, w, out):
    """One GEMM: out[M, NSH] = xT.T @ w, built on composable_matmul_tile_kernel
    with two tweaks over the stock matmul_tile_kernel:
      - W (kxn) loads issued on the ACT HWDGE ring (nc.scalar) so they run in
        parallel with xT loads / output stores on the SP ring;
      - eager eviction: each [128, 512] PSUM subtile is copied and DMAd to
        DRAM immediately, shrinking the kernel tail from ~6.5us to ~4us.
    """
    out_ap = out[:].rearrange("(mo mi) n -> mi mo n", mi=128)  # [128, 4, NSH]
    w_t = w[:].rearrange("(kt ks p) n -> kt p ks n", ks=4, p=128)
    xT_t = xT[:].rearrange("(kt ks p) m -> kt p ks m", ks=4, p=128)
    with ExitStack() as ctx:
        tc.swap_default_side()
        num_bufs = k_pool_min_bufs(w[:], max_tile_size=512)
        kxm_pool = ctx.enter_context(tc.tile_pool(name="kxm_pool", bufs=num_bufs))
        kxn_pool = ctx.enter_context(tc.tile_pool(name="kxn_pool", bufs=num_bufs))

        def kxm_producer(nc, md):
            t = kxm_pool.tile(
                [128, md.k_subtiles, md.m_tile], mybir.dt.float16, tag="kxm_t"
            )
            nc.sync.dma_start(out=t[:], in_=xT_t[md.k_tile_idx])
            return t[:]

        def kxn_producer(nc, md):
            t = kxn_pool.tile(
                [128, md.k_subtiles, md.n_tile], mybir.dt.float16, tag="kxn_t"
            )
            # W loads on the ACT HWDGE ring, parallel to the SP ring's
            # xT loads / output stores (measured best vs alternating rings)
            nc.scalar.dma_start(
                out=t[:], in_=w_t[md.k_tile_idx][:, :, ts(md.n_tile_idx, md.n_tile)]
            )
            return t[:]

        def reducer(nc, psum, sbuf, md):
            nc.any.tensor_copy(out=sbuf, in_=psum)
            dst = out_ap[
                :, md.m_tile_idx * md.m_subtiles + md.m_subtile_idx, md.n_subtile_slice
            ]
            nc.sync.dma_start(out=dst, in_=sbuf[:, 0, : md.n_subtile_slice_size])

        composable_matmul_tile_kernel(
            tc=tc,
            kxm_shape=ShapeInfo(pdims=((128, K // 128),), fdims=(M,)),
            kxn_shape=ShapeInfo(pdims=((128, K // 128),), fdims=(NSH,)),
            output_type=mybir.dt.float32,
            kxm_producer=kxm_producer,
            kxn_producer=kxn_producer,
            mxn_consumer=lambda nc, sbuf, md: None,  # reducer already stored
            mxn_subtile_reducer=reducer,
            cache_tiles=True,
        )


def _build(repeats=1):
    """Build the per-core Bass program. repeats>1 replicates the GEMM body
    inside one NEFF (used only for differential timing in test harnesses)."""
    global _NC
    if repeats == 1 and _NC is not None:
        return _NC
    nc = bass.Bass()
    xT = nc.declare_dram_parameter("xT", [K, M], mybir.dt.float16, isOutput=False)
    w = nc.declare_dram_parameter("w", [K, NSH], mybir.dt.float16, isOutput=False)
    out = nc.declare_dram_parameter("out", [M, NSH], mybir.dt.float32, isOutput=True)
    with tile.TileContext(nc) as tc:
        for _ in range(repeats):
            _gemm_body(nc, tc, xT, w, out)
    _split_sync_waits(nc)
    if repeats == 1:
        _NC = nc
    return nc


def _build_loop(repeats):
    """GEMM body wrapped in a hardware For_i loop (timing harness only)."""
    nc = bass.Bass()
    xT = nc.declare_dram_parameter("xT", [K, M], mybir.dt.float16, isOutput=False)
    w = nc.declare_dram_parameter("w", [K, NSH], mybir.dt.float16, isOutput=False)
    out = nc.declare_dram_parameter("out", [M, NSH], mybir.dt.float32, isOutput=True)
    with tile.TileContext(nc) as tc:
        with tc.For_i(0, repeats, 1):
            _gemm_body(nc, tc, xT, w, out)
    _split_sync_waits(nc)
    return nc


_RUNNER = None  # cached (fn, in_names, out_names, out_shapes) for repeat calls


def _make_runner(nc):
    """Build a reusable jitted shard_map executable for the SPMD kernel.

    Mirrors bass2jax.run_bass_via_pjrt (the @via_axon redirect target of
    run_bass_kernel_spmd) but caches the jitted function so repeated
    kernel() calls skip retracing/relowering.
    """
    import jax
    from jax.sharding import Mesh, NamedSharding, PartitionSpec
    from jax.experimental.shard_map import shard_map
    from concourse import bass2jax

    bass2jax.install_neuronx_cc_hook()
    partition_name = (
        nc.partition_id_tensor.name if nc.partition_id_tensor is not None else None
    )
    in_names, out_names, out_avals = [], [], []
    for alloc in nc.m.functions[0].allocations:
        if not isinstance(alloc, mybir.MemoryLocationSet):
            continue
        name = alloc.memorylocations[0].name
        if alloc.kind == "ExternalInput":
            if name != partition_name:
                in_names.append(name)
        elif alloc.kind == "ExternalOutput":
            out_names.append(name)
            out_avals.append(
                jax.core.ShapedArray(
                    tuple(alloc.tensor_shape), mybir.dt.np(alloc.dtype)
                )
            )
    n_params = len(in_names)
    all_names = list(in_names) + list(out_names)
    if partition_name is not None:
        all_names.append(partition_name)

    def _body(*args):
        operands = list(args)
        if partition_name is not None:
            operands.append(bass2jax.partition_id_tensor())
        return tuple(
            bass2jax._bass_exec_p.bind(
                *operands,
                out_avals=tuple(out_avals),
                in_names=tuple(all_names),
                out_names=tuple(out_names),
                lowering_input_output_aliases=(),
                sim_require_finite=True,
                sim_require_nnan=True,
                nc=nc,
            )
        )

    devices = jax.devices()[:NCORES]
    mesh = Mesh(np.asarray(devices), ("core",))
    spec = PartitionSpec("core")
    fn = jax.jit(
        shard_map(
            _body,
            mesh=mesh,
            in_specs=(spec,) * (n_params + len(out_names)),
            out_specs=(spec,) * len(out_names),
            check_rep=False,
        ),
        keep_unused=True,
    )
    sharding = NamedSharding(mesh, spec)
    return fn, sharding, in_names, out_names, out_avals


def _run_spmd_cached(nc, in_maps):
    """Run via a cached jitted executable; returns list of per-core out dicts."""
    global _RUNNER
    if _RUNNER is None:
        _RUNNER = _make_runner(nc)
    fn, sharding, in_names, out_names, out_avals = _RUNNER
    import jax

    concat_in = [
        jax.device_put(
            np.concatenate([np.asarray(m[name]) for m in in_maps], axis=0), sharding
        )
        for name in in_names
    ]
    concat_zero = [
        jax.device_put(
            np.zeros((NCORES * a.shape[0], *a.shape[1:]), a.dtype), sharding
        )
        for a in out_avals
    ]
    outs = fn(*concat_in, *concat_zero)
    return [
        {
            name: np.asarray(outs[i]).reshape(NCORES, *out_avals[i].shape)[c]
            for i, name in enumerate(out_names)
        }
        for c in range(NCORES)
    ]


def _run_spmd(nc, in_maps):
    """Run the SPMD kernel with defensive fallbacks:
    - primary: cached jitted executable (fast on repeat calls);
    - fallback: canonical run_bass_kernel_spmd, with the broken-NTFF-hook
      (missing antenv.axon_hooks) and transient-device-error cases handled.
    """
    import os

    try:
        results = _run_spmd_cached(nc, in_maps)
        return BassKernelResults(
            results=results,
            instructions_and_trace=None,
            profile_json=None,
            exec_time_ns=None,
        )
    except Exception:
        pass  # fall back to the canonical path below

    core_ids = list(range(NCORES))
    try:
        return run_bass_kernel_spmd(nc, in_maps, core_ids)
    except (ModuleNotFoundError, ImportError):
        os.environ["BASS_NEVER_TRACE"] = "1"
        return run_bass_kernel_spmd(nc, in_maps, core_ids)
    except Exception as e:  # transient NRT/axon failures
        msg = str(e)
        if "UNRECOVERABLE" in msg or "desynced" in msg or "UNAVAILABLE" in msg:
            return run_bass_kernel_spmd(nc, in_maps, core_ids)
        raise


def kernel(x, weight_int8, scales, bias):
    global LAST_RESULTS
    x = np.asarray(x, dtype=np.float32)
    weight_int8 = np.asarray(weight_int8)
    scales = np.asarray(scales, dtype=np.float32)
    bias = np.asarray(bias, dtype=np.float32)

    f16 = np.float16
    wdq = (
        (weight_int8.reshape(G, GROUP, N).astype(np.float32) * scales[:, None, :])
        .reshape(K, N)
        .astype(f16)
    )
    xT = np.ascontiguousarray(x.reshape(M, K).astype(f16).T)

    in_maps = [
        {"xT": xT, "w": np.ascontiguousarray(wdq[:, i * NSH : (i + 1) * NSH])}
        for i in range(NCORES)
    ]
    nc = _build()
    global LAST_IN_MAPS
    LAST_IN_MAPS = in_maps
    res = _run_spmd(nc, in_maps)
    LAST_RESULTS = res
    out = np.concatenate(
        [res.results[i]["out"] for i in range(NCORES)], axis=1
    ).astype(np.float32)
    out = out + bias[None, :]
    return out.reshape(B, S, N)
